# revision 1
# baseline (speedup 1.0000x reference)
"""Self-contained Trainium2 Bass kernel for nn_ChebNet_4320737100467.

ChebNet (K=2, two ChebConv layers + log_softmax) on a random graph with
N=100000 nodes, E=3200000 edges, sharded over 8 NeuronCores by destination
node. The separable symmetric normalization (w_e = -dis[row]*dis[col]) turns
edge aggregation into an unweighted gather-sum: rows are pre-scaled by dis
before projection and post-scaled by -dis after aggregation.

Per core: destination nodes are permuted into in-degree classes (K padded to
multiples of 4) so each node owns a fixed window of K gather slots laid out
[partition = node%128, free blocks = window]. Projected features are stored
in packed bf16 tables (4 nodes x 32 feats = one 256B row for layer A, 8 x 16
for layer B) so int16 dma_gather indices cover all 8*13952 permuted rows;
the low bits of the position select a 32/16-wide band after the gather via
one-hot masks, and a strided DVE reduction over (window x band) yields the
aggregate. Layer-to-layer halo exchange is a DRAM AllGather of the bf16
tables. Host only sorts/partitions topology and unpermutes the output.
"""

import numpy as np
import ml_dtypes
import jax
from jax.sharding import Mesh, PartitionSpec, NamedSharding
from jax.experimental.shard_map import shard_map

import concourse.bass as bass
import concourse.bacc as bacc
import concourse.tile as tile
import concourse.mybir as mybir
from concourse.bass2jax import _bass_exec_p, partition_id_tensor, install_neuronx_cc_hook

N = 100000
E = 3200000
NCORES = 8
NPD = 12500  # real nodes per device


def tile_plan(schedule):
    plan = []
    tile_i = 0
    blk0 = 0
    for kv, t in schedule:
        for _ in range(t):
            calls = []
            rem = kv
            while rem > 0:
                calls.append(min(8, rem))
                rem -= 8
            plan.append((kv, tile_i, blk0, calls))
            tile_i += 1
            blk0 += kv
    return plan


def preprocess(edge_index: np.ndarray):
    row = edge_index[0].astype(np.int64)
    col = edge_index[1].astype(np.int64)
    deg_full = np.bincount(row, minlength=N)
    dis_full = np.where(
        deg_full > 0, 1.0 / np.sqrt(np.maximum(deg_full, 1.0)), 0.0
    ).astype(np.float32)

    dev = row // NPD
    per = []
    for d in range(NCORES):
        m = dev == d
        per.append((row[m] - d * NPD, col[m]))

    Ks, perms = [], []
    for d in range(NCORES):
        r_loc, _ = per[d]
        degd = np.bincount(r_loc, minlength=NPD)
        K = np.maximum(4, ((degd + 3) // 4) * 4)
        perms.append(np.argsort(K, kind="stable"))
        Ks.append(K)

    kvals = sorted(set(int(k) for K in Ks for k in np.unique(K)))
    schedule = []
    for kv in kvals:
        cnt = max(int((K == kv).sum()) for K in Ks)
        t = (cnt + 127) // 128
        if t > 0:
            schedule.append((kv, t))
    ntiles = sum(t for _, t in schedule)
    nblk = sum(kv * t for kv, t in schedule)
    rows = ntiles * 128
    plan = tile_plan(schedule)
    ncalls = sum(len(p[3]) for p in plan)

    node_of_pos = np.full((NCORES, rows), -1, np.int64)
    for d in range(NCORES):
        K = Ks[d]
        pos = 0
        for kv, t in schedule:
            ids = perms[d][K[perms[d]] == kv]
            node_of_pos[d, pos : pos + len(ids)] = ids
            pos += t * 128
    pos_of_node = np.full((NCORES, NPD), -1, np.int64)
    for d in range(NCORES):
        real = node_of_pos[d] >= 0
        pos_of_node[d, node_of_pos[d][real]] = np.nonzero(real)[0]

    table_rows_A = NCORES * rows // 4
    table_rows_B = NCORES * rows // 8
    assert table_rows_A <= 32768 and table_rows_B <= 32768

    idxA = np.zeros((NCORES, 128, ncalls * 64), np.int16)
    idxB = np.zeros((NCORES, 128, ncalls * 64), np.int16)
    bandA = np.full((NCORES, 128, nblk), 255, np.int64)
    bandB = np.full((NCORES, 128, nblk), 255, np.int64)
    grids = np.full((NCORES, 128, nblk), -1, np.int64)

    for d in range(NCORES):
        r_loc, c_glob = per[d]
        order = np.argsort(r_loc, kind="stable")
        r_s = r_loc[order]
        estart = np.searchsorted(r_s, np.arange(NPD + 1))
        gpos_col = (c_glob // NPD) * rows + pos_of_node[c_glob // NPD, c_glob % NPD]
        gpos_s = gpos_col[order]

        grid = np.full((128, nblk), -1, np.int64)
        for kv, tile_i, blk0, _calls in plan:
            nodes = node_of_pos[d, tile_i * 128 : (tile_i + 1) * 128]
            for p in range(128):
                nd = nodes[p]
                if nd < 0:
                    continue
                s0, s1 = estart[nd], estart[nd + 1]
                assert s1 - s0 <= kv
                grid[p, blk0 : blk0 + (s1 - s0)] = gpos_s[s0:s1]
        grids[d] = grid

        valid = grid >= 0
        ia = np.where(valid, grid >> 2, 0)
        ib = np.where(valid, grid >> 3, 0)
        bandA[d] = np.where(valid, grid & 3, 255)
        bandB[d] = np.where(valid, grid & 7, 255)

        call_ix = 0
        for kv, tile_i, blk0, calls in plan:
            done = 0
            for nb in calls:
                sub_a = ia[:, blk0 + done : blk0 + done + nb]  # [128, nb]
                sub_b = ib[:, blk0 + done : blk0 + done + nb]
                flat_a = sub_a.T.reshape(-1)  # i = blk*128 + p
                flat_b = sub_b.T.reshape(-1)
                wrapped_a = np.tile(flat_a.reshape(-1, 16).T, (8, 1))  # [128, nb*8]
                wrapped_b = np.tile(flat_b.reshape(-1, 16).T, (8, 1))
                idxA[d, :, call_ix * 64 : call_ix * 64 + nb * 8] = wrapped_a
                idxB[d, :, call_ix * 64 : call_ix * 64 + nb * 8] = wrapped_b
                done += nb
                call_ix += 1
        assert call_ix == ncalls

    assert int(idxA.max()) < table_rows_A and int(idxB.max()) < table_rows_B

    return dict(
        dis_full=dis_full,
        schedule=schedule,
        plan=plan,
        ntiles_total=ntiles,
        nblk_total=nblk,
        ncalls=ncalls,
        rows_per_dev=rows,
        node_of_pos=node_of_pos,
        idxA=idxA,
        idxB=idxB,
        bandA=bandA,
        bandB=bandB,
        grids=grids,
    )


def build_core_inputs(x, dis_full, meta, d):
    nop = meta["node_of_pos"][d]
    rows = len(nop)
    xp = np.zeros((rows, 128), np.float32)
    disp = np.zeros(rows, np.float32)
    real = nop >= 0
    gids = nop[real] + d * NPD
    xp[real] = np.asarray(x)[gids]
    disp[real] = dis_full[gids]
    nt = meta["ntiles_total"]
    dis2d = np.ascontiguousarray(disp.reshape(nt, 128).T)
    return xp, dis2d


# ---------------- kernel builder ----------------
F32 = mybir.dt.float32
BF16 = mybir.dt.bfloat16
I16 = mybir.dt.int16
U8 = mybir.dt.uint8
AX = mybir.AxisListType
ALU = mybir.AluOpType
ACT = mybir.ActivationFunctionType

NCORES = 8
IDX_CHUNK = 32  # calls per idx-chunk load


def tile_plan(schedule):
    """[(K, tile_i, blk0, [call num_idx blocks ...])]"""
    plan = []
    tile_i = 0
    blk0 = 0
    for kv, t in schedule:
        for _ in range(t):
            calls = []
            rem = kv
            while rem > 0:
                calls.append(min(8, rem))
                rem -= 8
            plan.append((kv, tile_i, blk0, calls))
            tile_i += 1
            blk0 += kv
    return plan


def build(meta):
    schedule = meta["schedule"]
    ntiles = meta["ntiles_total"]
    rows = meta["rows_per_dev"]
    plan = tile_plan(schedule)
    ncalls = sum(len(p[3]) for p in plan)
    kmax = max(kv for kv, _ in schedule)
    nblk = meta["nblk_total"]
    ta_rows = NCORES * rows // 4
    tb_rows = NCORES * rows // 8

    nc = bacc.Bacc(
        "TRN2",
        target_bir_lowering=False,
        debug=False,
        num_devices=NCORES,
    )
    x_in = nc.declare_dram_parameter("x_perm", [rows, 128], F32, isOutput=False)
    dis_in = nc.declare_dram_parameter("dis2d", [128, ntiles], F32, isOutput=False)
    w01a_in = nc.declare_dram_parameter("W01a", [128, 64], F32, isOutput=False)
    w01b_in = nc.declare_dram_parameter("W01b", [32, 32], F32, isOutput=False)
    ba_in = nc.declare_dram_parameter("ba2", [128, 32], F32, isOutput=False)
    bb_in = nc.declare_dram_parameter("bb2", [128, 16], F32, isOutput=False)
    eye_in = nc.declare_dram_parameter("eye", [128, 128], F32, isOutput=False)
    idxa_in = nc.declare_dram_parameter(
        "idxA2", [128, ncalls * 64], I16, isOutput=False
    )
    idxb_in = nc.declare_dram_parameter(
        "idxB2", [128, ncalls * 64], I16, isOutput=False
    )
    banda_in = nc.declare_dram_parameter("bandA", [128, nblk], BF16, isOutput=False)
    bandb_in = nc.declare_dram_parameter("bandB", [128, nblk], BF16, isOutput=False)
    out_ext = nc.declare_dram_parameter("out_perm", [rows, 16], F32, isOutput=True)

    pa_slice = nc.dram_tensor("pa_slice", [rows // 4, 128], BF16)
    pb_slice = nc.dram_tensor("pb_slice", [rows // 8, 128], BF16)
    pa_table = nc.dram_tensor("pa_table", [ta_rows, 128], BF16, addr_space="Shared")
    pb_table = nc.dram_tensor("pb_table", [tb_rows, 128], BF16, addr_space="Shared")

    groups = [list(range(NCORES))]

    with tile.TileContext(nc) as tc:
        with (
            tc.tile_pool(name="const", bufs=1) as cpool,
            tc.tile_pool(name="work", bufs=3) as wpool,
            tc.tile_pool(name="gath", bufs=2) as gpool,
            tc.tile_pool(name="psum", bufs=2, space="PSUM") as ppool,
            tc.tile_pool(name="psum1", bufs=2, space="PSUM") as ppool1,
        ):
            # ---- constants / residents
            w01a = cpool.tile([128, 64], F32)
            nc.sync.dma_start(out=w01a[:], in_=w01a_in[:])
            w01b = cpool.tile([32, 32], F32)
            nc.sync.dma_start(out=w01b[:], in_=w01b_in[:])
            ba_sb = cpool.tile([128, 32], F32)
            nc.sync.dma_start(out=ba_sb[:], in_=ba_in[:])
            bb_sb = cpool.tile([128, 16], F32)
            nc.sync.dma_start(out=bb_sb[:], in_=bb_in[:])
            eye = cpool.tile([128, 128], F32)
            nc.sync.dma_start(out=eye[:], in_=eye_in[:])
            dis_sb = cpool.tile([128, ntiles], F32)
            nc.sync.dma_start(out=dis_sb[:], in_=dis_in[:])
            negdis = cpool.tile([128, ntiles], F32)
            nc.vector.tensor_scalar_mul(negdis[:], dis_sb[:], -1.0)
            q_sb = cpool.tile([128, ntiles * 32], F32)
            hwb_sb = cpool.tile([128, ntiles * 16], F32)
            banda_sb = cpool.tile([128, nblk], BF16)
            nc.sync.dma_start(out=banda_sb[:], in_=banda_in[:])
            bandb_sb = cpool.tile([128, nblk], BF16)
            nc.sync.dma_start(out=bandb_sb[:], in_=bandb_in[:])

            # ---- S1: projections over own tiles
            for t in range(ntiles):
                xt = wpool.tile([128, 128], F32, tag="xtile")
                nc.sync.dma_start(out=xt[:], in_=x_in[t * 128 : (t + 1) * 128, :])
                xT_ps = ppool.tile([128, 128], F32, tag="xT")
                nc.tensor.transpose(out=xT_ps[:], in_=xt[:], identity=eye[:])
                xT = wpool.tile([128, 128], F32, tag="xT_sb")
                nc.vector.tensor_copy(out=xT[:], in_=xT_ps[:])
                p01 = ppool1.tile([128, 64], F32, tag="p01")
                nc.tensor.matmul(p01[:], lhsT=xT[:], rhs=w01a[:], start=True, stop=True)
                nc.vector.tensor_copy(
                    out=q_sb[:, t * 32 : (t + 1) * 32], in_=p01[:, 0:32]
                )
                pa_bf = wpool.tile([128, 32], BF16, tag="pa_bf")
                nc.vector.tensor_scalar_mul(
                    pa_bf[:], p01[:, 32:64], dis_sb[:, t : t + 1]
                )
                nc.sync.dma_start(
                    out=pa_slice[t * 32 : (t + 1) * 32, :].rearrange(
                        "r (g f) -> (r g) f", g=4
                    ),
                    in_=pa_bf[:],
                )

            # ---- S2: allgather pa
            nc.gpsimd.collective_compute(
                "AllGather",
                ALU.bypass,
                replica_groups=groups,
                ins=[pa_slice[:, :]],
                outs=[pa_table[:, :]],
            )

            # ---- S3 + S5 helper
            def edge_phase(
                layer,
                table,
                idx_in,
                band_sb,
                nband,
                fw,
            ):
                call_ix = 0
                chunk = None
                for kv, tile_i, blk0, calls in plan:
                    gt = gpool.tile([128, kmax, 128], BF16, tag=f"g{layer}")
                    done = 0
                    for nb in calls:
                        if call_ix % IDX_CHUNK == 0:
                            chunk = wpool.tile(
                                [128, IDX_CHUNK * 64], I16, tag=f"idx{layer}"
                            )
                            c0 = call_ix * 64
                            cols = min(IDX_CHUNK * 64, ncalls * 64 - c0)
                            nc.sync.dma_start(
                                out=chunk[:, 0:cols], in_=idx_in[:, c0 : c0 + cols]
                            )
                        off = (call_ix % IDX_CHUNK) * 64
                        nidx = nb * 128
                        nc.gpsimd.dma_gather(
                            gt[:, done : done + nb, :],
                            table[:, :],
                            chunk[:, off : off + nidx // 16],
                            nidx,
                            nidx,
                            128,
                            single_packet=False,
                        )
                        done += nb
                        call_ix += 1
                    # masks: [128, kv, nband] bf16 one-hot of band ids
                    mask = wpool.tile([128, kmax * nband], BF16, tag=f"m{layer}")
                    for j in range(nband):
                        nc.vector.tensor_scalar(
                            out=mask[:].rearrange("p (k j) -> p k j", j=nband)[
                                :, 0:kv, j
                            ],
                            in0=band_sb[:, blk0 : blk0 + kv],
                            scalar1=float(j),
                            scalar2=None,
                            op0=ALU.is_equal,
                        )
                    # masked multiply in place: gt viewed [p, kv*nband, fw]
                    gview = gt[:, 0:kv, :].rearrange("p k (j f) -> p (k j) f", f=fw)
                    nc.vector.tensor_tensor(
                        out=gview,
                        in0=gview,
                        in1=mask[:, 0 : kv * nband].to_broadcast(
                            [128, kv * nband, fw]
                        ),
                        op=ALU.mult,
                    )
                    # reduce over (k j) with feat innermost-kept:
                    # gt free layout offset = k*128 + j*fw + f
                    red_in = gt[:, 0:kv, :].rearrange(
                        "p k (j f) -> p f k j", f=fw
                    )
                    tred = wpool.tile([128, fw], F32, tag=f"t{layer}")
                    nc.vector.tensor_reduce(tred[:], red_in, AX.XY, ALU.add)
                    yield kv, tile_i, tred

            # ---- S3: layer A edge phase + h + layer B projections
            for kv, t, t1 in edge_phase("a", pa_table, idxa_in, banda_sb, 4, 32):
                z = wpool.tile([128, 32], F32, tag="z1")
                nc.vector.tensor_scalar_mul(z[:], t1[:], negdis[:, t : t + 1])
                nc.vector.tensor_tensor(
                    out=z[:], in0=z[:], in1=q_sb[:, t * 32 : (t + 1) * 32], op=ALU.add
                )
                nc.vector.tensor_tensor(
                    out=z[:], in0=z[:], in1=ba_sb[:], op=ALU.add
                )
                h = wpool.tile([128, 32], F32, tag="h")
                nc.scalar.activation(h[:], z[:], ACT.Relu)
                hT_ps = ppool.tile([32, 128], F32, tag="hT")
                nc.tensor.transpose(out=hT_ps[:], in_=h[:], identity=eye[:])
                hT = wpool.tile([32, 128], F32, tag="hT_sb")
                nc.vector.tensor_copy(out=hT[:], in_=hT_ps[:])
                pb01 = ppool1.tile([128, 32], F32, tag="pb01")
                nc.tensor.matmul(
                    pb01[:], lhsT=hT[:], rhs=w01b[:], start=True, stop=True
                )
                nc.vector.tensor_copy(
                    out=hwb_sb[:, t * 16 : (t + 1) * 16], in_=pb01[:, 0:16]
                )
                pb_bf = wpool.tile([128, 16], BF16, tag="pb_bf")
                nc.vector.tensor_scalar_mul(
                    pb_bf[:], pb01[:, 16:32], dis_sb[:, t : t + 1]
                )
                nc.sync.dma_start(
                    out=pb_slice[t * 16 : (t + 1) * 16, :].rearrange(
                        "r (g f) -> (r g) f", g=8
                    ),
                    in_=pb_bf[:],
                )

            # ---- S4: allgather pb
            nc.gpsimd.collective_compute(
                "AllGather",
                ALU.bypass,
                replica_groups=groups,
                ins=[pb_slice[:, :]],
                outs=[pb_table[:, :]],
            )

            # ---- S5: layer B edge phase + tail
            for kv, t, t2 in edge_phase("b", pb_table, idxb_in, bandb_sb, 8, 16):
                z = wpool.tile([128, 16], F32, tag="z2")
                nc.vector.tensor_scalar_mul(z[:], t2[:], negdis[:, t : t + 1])
                nc.vector.tensor_tensor(
                    out=z[:], in0=z[:], in1=hwb_sb[:, t * 16 : (t + 1) * 16], op=ALU.add
                )
                nc.vector.tensor_tensor(
                    out=z[:], in0=z[:], in1=bb_sb[:], op=ALU.add
                )
                mx = wpool.tile([128, 1], F32, tag="mx")
                nc.vector.tensor_reduce(mx[:], z[:], AX.X, ALU.max)
                zc = wpool.tile([128, 16], F32, tag="zc")
                nc.vector.tensor_scalar(
                    out=zc[:], in0=z[:], scalar1=mx[:, 0:1], scalar2=None,
                    op0=ALU.subtract,
                )
                ex = wpool.tile([128, 16], F32, tag="ex")
                nc.scalar.activation(ex[:], zc[:], ACT.Exp)
                sm = wpool.tile([128, 1], F32, tag="sm")
                nc.vector.tensor_reduce(sm[:], ex[:], AX.X, ALU.add)
                ls = wpool.tile([128, 1], F32, tag="ls")
                nc.scalar.activation(ls[:], sm[:], ACT.Ln)
                ot = wpool.tile([128, 16], F32, tag="ot")
                nc.vector.tensor_scalar(
                    out=ot[:], in0=zc[:], scalar1=ls[:, 0:1], scalar2=None,
                    op0=ALU.subtract,
                )
                nc.sync.dma_start(
                    out=out_ext[t * 128 : (t + 1) * 128, :], in_=ot[:]
                )

    nc.finalize()
    return nc


# ---------------- runner ----------------
class SpmdRunner:
    def __init__(self, nc: bass.Bass, n_cores: int):
        install_neuronx_cc_hook()
        self.nc = nc
        self.n_cores = n_cores
        partition_name = nc.partition_id_tensor.name if nc.partition_id_tensor else None
        in_names, out_names, out_avals = [], [], []
        for alloc in nc.m.functions[0].allocations:
            if not isinstance(alloc, mybir.MemoryLocationSet):
                continue
            name = alloc.memorylocations[0].name
            if alloc.kind == "ExternalInput":
                if name != partition_name:
                    in_names.append(name)
            elif alloc.kind == "ExternalOutput":
                out_names.append(name)
                out_avals.append(
                    jax.core.ShapedArray(
                        tuple(alloc.tensor_shape), mybir.dt.np(alloc.dtype)
                    )
                )
        self.in_names = list(in_names)
        self.out_names = out_names
        self.out_avals = out_avals
        n_params = len(in_names)
        all_names = in_names + out_names
        if partition_name is not None:
            all_names.append(partition_name)
        self.partition_name = partition_name

        def _body(*args):
            operands = list(args)
            if partition_name is not None:
                operands.append(partition_id_tensor())
            return tuple(
                _bass_exec_p.bind(
                    *operands,
                    out_avals=tuple(out_avals),
                    in_names=tuple(all_names),
                    out_names=tuple(out_names),
                    lowering_input_output_aliases=(),
                    sim_require_finite=True,
                    sim_require_nnan=True,
                    nc=nc,
                )
            )

        devices = jax.devices()[:n_cores]
        assert len(devices) == n_cores
        self.mesh = Mesh(np.asarray(devices), ("core",))
        n_io = n_params + len(out_names)
        self.fn = jax.jit(
            shard_map(
                _body,
                mesh=self.mesh,
                in_specs=(PartitionSpec("core"),) * n_io,
                out_specs=(PartitionSpec("core"),) * len(out_names),
                check_rep=False,
            ),
            keep_unused=True,
        )
        self.sharding = NamedSharding(self.mesh, PartitionSpec("core"))
        self._dev_in = None

    def put_inputs(self, in_maps: list[dict[str, np.ndarray]]):
        """Upload per-core inputs (list of dicts) to device, concatenated on
        axis 0 with 'core' sharding."""
        assert len(in_maps) == self.n_cores
        concat = [
            np.concatenate([np.asarray(m[name]) for m in in_maps], axis=0)
            for name in self.in_names
        ]
        zeros = [
            np.zeros((self.n_cores * a.shape[0], *a.shape[1:]), a.dtype)
            for a in self.out_avals
        ]
        self._dev_in = [jax.device_put(a, self.sharding) for a in concat + zeros]
        return self

    def run(self):
        outs = self.fn(*self._dev_in)
        jax.block_until_ready(outs)
        return outs

    def results(self, outs) -> list[dict[str, np.ndarray]]:
        res = []
        for c in range(self.n_cores):
            d = {}
            for i, name in enumerate(self.out_names):
                full = np.asarray(outs[i])
                per = full.reshape(self.n_cores, *self.out_avals[i].shape)
                d[name] = per[c]
            res.append(d)
        return res

    def time_runs(self, reps=5):
        walls = []
        for _ in range(reps):
            t = time.time()
            self.run()
            walls.append(time.time() - t)
        return min(walls), walls


# ---------------- driver / entry point ----------------
def make_in_maps(inputs, meta):
    x = np.asarray(inputs["x"], np.float32)
    W0a = np.asarray(inputs["W0a"], np.float32)
    W1a = np.asarray(inputs["W1a"], np.float32)
    W0b = np.asarray(inputs["W0b"], np.float32)
    W1b = np.asarray(inputs["W1b"], np.float32)
    ba = np.asarray(inputs["ba"], np.float32)
    bb = np.asarray(inputs["bb"], np.float32)
    w01a = np.concatenate([W0a, W1a], axis=1)  # [128, 64]
    w01b = np.concatenate([W0b, W1b], axis=1)  # [32, 32]
    eye = np.eye(128, dtype=np.float32)
    bf = ml_dtypes.bfloat16
    in_maps = []
    for d in range(NCORES):
        xp, dis2d = build_core_inputs(x, meta["dis_full"], meta, d)
        in_maps.append(
            dict(
                x_perm=xp,
                dis2d=dis2d,
                W01a=w01a,
                W01b=w01b,
                ba2=np.tile(ba.reshape(1, 32), (128, 1)),
                bb2=np.tile(bb.reshape(1, 16), (128, 1)),
                eye=eye,
                idxA2=meta["idxA"][d],
                idxB2=meta["idxB"][d],
                bandA=meta["bandA"][d].astype(np.float32).astype(bf),
                bandB=meta["bandB"][d].astype(np.float32).astype(bf),
            )
        )
    return in_maps


def unpermute(outs, meta):
    """outs: list of per-core out_perm [rows, 16] -> [N, 16]."""
    out_full = np.zeros((N, 16), np.float32)
    for d in range(NCORES):
        nop = meta["node_of_pos"][d]
        real = nop >= 0
        out_full[nop[real] + d * NPD] = np.asarray(outs[d])[np.nonzero(real)[0]]
    return out_full


_CACHE = {}


def kernel(**inputs) -> np.ndarray:
    edge_index = np.asarray(inputs["edge_index"])
    key = edge_index.tobytes()[:4096]
    if key not in _CACHE:
        meta = preprocess(edge_index)
        nc = build(meta)
        runner = SpmdRunner(nc, NCORES)
        _CACHE[key] = (meta, runner)
    meta, runner = _CACHE[key]
    in_maps = make_in_maps(inputs, meta)
    runner.put_inputs(in_maps)
    outs = runner.run()
    res = runner.results(outs)
    return unpermute([res[d]["out_perm"] for d in range(NCORES)], meta)

